# revision 10
# baseline (speedup 1.0000x reference)
"""Class-conditional linear dispatch (MoE routing) on 8 trn2 NeuronCores.

y[i] = x[i] @ W[cls[i]] + b[cls[i]]   with B=8192, D=512, C=16 classes.

Strategy: expert-parallel with HOST-side dispatch. The host routes rows to
classes (the all-to-all), assigns 2 classes per core (largest paired with
smallest so per-class capacities stay tight), and uploads each core's rows
pre-sorted AND pre-transposed (x^T layout) plus a pre-broadcast bias. The
device kernel is a dense pipeline: stream x^T tiles (Pool/SWDGE queue) and
the 2 weight matrices (split across the SP+Activation HWDGE queues) into
SBUF, run K-chunked fp32r matmuls on the PE (x^T chunks stationary, W
moving), add bias on DVE/Pool straight out of PSUM, and store each 128-row
tile on alternating queues. Scratch matmuls at t=0 lift the PE out of its
throttled (HAM cold) clock before real data lands. No gathers and no
on-device transposes. The host scatters the compact per-core outputs back
to original row order.
"""

import os
import sys

import numpy as np

_TRN_REPO = "/opt/trn_rl_repo"
if _TRN_REPO not in sys.path:
    sys.path.insert(0, _TRN_REPO)

B, D_IN, D_OUT, C, NCORES = 8192, 512, 512, 16, 8
CPL = C // NCORES  # classes per core
KC = D_IN // 128  # contraction chunks of 128

# Set by callers that want profiling; results stashed in LAST_RESULT.
TRACE = False
LAST_RESULT = None

BEST_VARIANT = {"n_warm": 8}


def _xt_bounds(T, xt_split):
    """Cumulative row-tile boundaries for the x^T load groups."""
    bounds = [0]
    for n in xt_split:
        if bounds[-1] >= T:
            break
        bounds.append(min(T, bounds[-1] + n))
    while bounds[-1] < T:
        bounds.append(min(T, bounds[-1] + xt_split[-1]))
    return bounds


def build_nc(
    cap_a: int,
    cap_b: int,
    *,
    n_warm: int = 6,
    xt_split=(2, 3, 2, 2),
    psum_bufs: int = 6,
    ysb_bufs: int = 6,
    loop_reps: int = 1,
):
    """Per-core Bass program. cap_a/cap_b: rows (multiple of 128) for the
    core's first/second class slot. Row-tiles 0..cap_a/128-1 use slot 0.

    xt_split: row-tiles per x^T DMA group (all on the Pool queue).
    n_warm: scratch matmuls at t=0 to warm the PE clock.
    """
    import concourse.bacc as bacc
    import concourse.mybir as mybir
    from concourse import tile

    f32 = mybir.dt.float32
    f32r = mybir.dt.float32r
    R = cap_a + cap_b
    T = R // 128
    TA = cap_a // 128

    nc = bacc.Bacc(
        "TRN2",
        target_bir_lowering=False,
        debug=False,
    )
    xt_d = nc.dram_tensor("xt", [D_IN, R], f32r, kind="ExternalInput")
    w_d = nc.dram_tensor("wl", [CPL, D_IN, D_OUT], f32r, kind="ExternalInput")
    b_d = nc.dram_tensor("bbc", [128, CPL * D_OUT], f32, kind="ExternalInput")
    y_d = nc.dram_tensor("y", [R, D_OUT], f32, kind="ExternalOutput")

    slot_of = [0 if t < TA else 1 for t in range(T)]
    bounds = _xt_bounds(T, xt_split)

    with tile.TileContext(nc) as tc:
        from contextlib import nullcontext

        with (
            tc.tile_pool(name="const", bufs=1) as cpool,
            tc.tile_pool(name="pswarm", bufs=1, space="PSUM") as wpool,
            tc.tile_pool(name="psy", bufs=psum_bufs, space="PSUM") as psyp,
            tc.tile_pool(name="ysb", bufs=ysb_bufs) as ypool,
            tc.For_i(0, loop_reps, 1) if loop_reps > 1 else nullcontext(),
        ):
            # -- PE warmup: scratch matmuls, earliest possible -------------
            if n_warm:
                warm_sb = cpool.tile([128, 128], f32, tag="warm")
                nc.vector.memset(warm_sb[:], 0.0)
                warm_ps = wpool.tile([128, D_OUT], f32, tag="warmps")
                for i in range(n_warm):
                    nc.tensor.matmul(
                        warm_ps[:, :128],
                        warm_sb[:],
                        warm_sb[:],
                        start=True,
                        stop=True,
                    )

            # -- loads -----------------------------------------------------
            # W: both classes split in halves across Act+SP so every chunk
            # lands by ~2x one half-transfer. Issued first on both queues.
            w_sb = cpool.tile([128, CPL * KC, D_OUT], f32r, tag="w")
            w_view = [
                w_d[c].rearrange("(kc p) n -> p kc n", p=128) for c in range(CPL)
            ]
            nc.scalar.dma_start(w_sb[:, 0:2, :], w_view[0][:, 0:2, :])
            nc.sync.dma_start(w_sb[:, 2:4, :], w_view[0][:, 2:4, :])
            nc.sync.dma_start(w_sb[:, KC : KC + 2, :], w_view[1][:, 0:2, :])
            nc.scalar.dma_start(w_sb[:, KC + 2 : KC + 4, :], w_view[1][:, 2:4, :])
            # (bias halves are issued below, right after these)

            # x^T groups: all on the Pool/SWDGE queue, growing sizes so the
            # first tiles land early while the PE chews through them.
            xt_sb = cpool.tile([128, KC, R], f32r, tag="xt")
            xt_view = xt_d.rearrange("(kc p) r -> p kc r", p=128)
            for g in range(len(bounds) - 1):
                nc.gpsimd.dma_start(
                    xt_sb[:, :, bounds[g] * 128 : bounds[g + 1] * 128],
                    xt_view[:, :, bounds[g] * 128 : bounds[g + 1] * 128],
                )

            # bias, pre-broadcast on host: [128, CPL*D_OUT], split across
            # both HWDGE queues right behind the first W halves
            b_bc = cpool.tile([128, CPL, D_OUT], f32, tag="bbc")
            b_view = b_d.rearrange("p (c n) -> p c n", c=CPL)
            nc.sync.dma_start(b_bc[:, 0, :], b_view[:, 0, :])
            nc.scalar.dma_start(b_bc[:, 1, :], b_view[:, 1, :])

            # -- compute + store -------------------------------------------
            for t in range(T):
                c = slot_of[t]
                y_ps = psyp.tile([128, D_OUT], f32)
                for k in range(KC):
                    nc.tensor.matmul(
                        y_ps[:],
                        xt_sb[:, k, t * 128 : (t + 1) * 128],
                        w_sb[:, c * KC + k, :],
                        start=(k == 0),
                        stop=(k == KC - 1),
                    )
                # GPSIMD cannot access PSUM (BIR verifier), so all bias
                # adds (which double as PSUM evacuation) run on DVE.
                y_sb = ypool.tile([128, D_OUT], f32)
                nc.vector.tensor_add(y_sb[:], y_ps[:], b_bc[:, c, :])
                store_eng = nc.sync if t % 2 == 0 else nc.scalar
                store_eng.dma_start(y_d[t * 128 : (t + 1) * 128, :], y_sb[:])

    nc.compile()
    return nc


def _route(cls_np: np.ndarray):
    """Host-side dispatch: per-class row lists, class->core assignment and
    per-slot capacities.

    Pair the k-th largest class with the k-th smallest so the max count in
    each slot (which sets the uniform capacity) stays tight.
    Returns (assign, rows_per_class, cap_a, cap_b) where assign[k] =
    (class for slot A, class for slot B) of core k.
    """
    order = np.argsort(cls_np, kind="stable")
    counts = np.bincount(cls_np, minlength=C)
    starts = np.zeros(C + 1, dtype=np.int64)
    starts[1:] = np.cumsum(counts)
    rows_per_class = [order[starts[c] : starts[c + 1]] for c in range(C)]

    by_count = np.argsort(counts, kind="stable")[::-1]  # desc
    assign = [
        (int(by_count[k]), int(by_count[C - 1 - k])) for k in range(NCORES)
    ]
    ceil128 = lambda n: max(1, -(-int(n) // 128))
    cap_a = 128 * max(ceil128(counts[a]) for a, _ in assign)
    cap_b = 128 * max(ceil128(counts[b]) for _, b in assign)
    return assign, rows_per_class, cap_a, cap_b


def make_in_maps(x, W, b, assign, rows_per_class, cap_a, cap_b):
    """Per-core input maps: class-sorted, padded, transposed x plus the
    core's two weight matrices and pre-broadcast biases (slot A first)."""
    R = cap_a + cap_b
    in_maps = []
    for ca, cb in assign:
        xs = np.zeros((R, D_IN), dtype=np.float32)
        ra, rb = rows_per_class[ca], rows_per_class[cb]
        xs[: len(ra)] = x[ra]
        xs[cap_a : cap_a + len(rb)] = x[rb]
        bbc = np.broadcast_to(
            b[[ca, cb]].reshape(1, CPL * D_OUT), (128, CPL * D_OUT)
        )
        in_maps.append(
            {
                "xt": np.ascontiguousarray(xs.T),
                "wl": np.ascontiguousarray(W[[ca, cb]]),
                "bbc": np.ascontiguousarray(bbc),
            }
        )
    return in_maps


def kernel(x, cls, W, b):
    from concourse.bass_utils import run_bass_kernel_spmd

    global LAST_RESULT
    x = np.ascontiguousarray(np.asarray(x), dtype=np.float32)
    cls_np = np.asarray(cls).astype(np.int64).ravel()
    W = np.ascontiguousarray(np.asarray(W), dtype=np.float32)
    b = np.ascontiguousarray(np.asarray(b), dtype=np.float32)

    assign, rows_per_class, cap_a, cap_b = _route(cls_np)
    in_maps = make_in_maps(x, W, b, assign, rows_per_class, cap_a, cap_b)
    nc = build_nc(cap_a, cap_b, **BEST_VARIANT)
    res = run_bass_kernel_spmd(
        nc,
        in_maps,
        core_ids=list(range(NCORES)),
        trace=TRACE,
        trace_cores=list(range(NCORES)) if TRACE else None,
    )
    LAST_RESULT = res

    out = np.empty((B, D_OUT), dtype=np.float32)
    for k, (ca, cb) in enumerate(assign):
        y = res.results[k]["y"]
        ra, rb = rows_per_class[ca], rows_per_class[cb]
        out[ra] = y[: len(ra)]
        out[rb] = y[cap_a : cap_a + len(rb)]
    return out
